# revision 5
# baseline (speedup 1.0000x reference)
"""Skip-gram negative-sampling loss on 8 Trainium2 NeuronCores.

Strategy v2 (data-parallel over batch, hint-conformant):
  - Each core handles 2048 batch rows and 512 hierarchy pairs.
  - Row-major pair layout: block b (of 16) covers batch rows
    [128b, 128b+128); pair (row, j) sits at partition row%128, slot j
    (j<10: pos, j>=10: neg). The input-row operand is gathered ONCE per
    core (2048 rows) and broadcast (stride-0) across the 60 slots by the
    DVE multiply, eliminating half of the baseline's per-pair gathers.
  - dma_gather needs int16 indices into a single offset-0 source. Rather
    than range-bucketing the vocab (which forces stream sorting and
    padding), each 512-row sub-batch gets a host-packed DEDUP'd table of
    the distinct out_embed rows it references (~26.5k < 32767 rows), so
    every block gather is one single-source call with zero padding.
  - Hierarchy pairs use a per-core packed in_embed table (<=1024 rows);
    one 1024-row gather yields both sides pair-aligned.
  - Per-block compute: prod = R * I[:,b,:] (bf16, broadcast), add
    halves, reduce -> dots [128, 16, 60] f32. Tail: softplus via
    max(v,0)+ln(1+exp(-|v|)) with the pos/neg sign handled by two
    tensor_scalar variants on slot ranges; hier slots: sub, square,
    reduce. Output per core: [128, 2] partial sums, summed on host in
    f64. No padding corrections needed: every gathered pair is real.
"""

import numpy as np
import ml_dtypes

import concourse.bacc as bacc
import concourse.tile as tile
from concourse import mybir

# Problem shape (hardcoded per contract).
B = 16384
V = 100000
D = 128
C = 10
NEG = 50
PH = 4096
NCORES = 8
P = 128

BL = B // NCORES          # 2048 batch rows per core
HLC = PH // NCORES        # 512 hierarchy pairs per core
NBLK = BL // P            # 16 blocks of 128 rows
SLOTS = C + NEG           # 60 pairs per batch row
SUB = 512                 # rows per dedup sub-batch (keeps tables < int16)
NSUB = BL // SUB          # 4 sub-batches per core
BPS = SUB // P            # 4 blocks per sub-batch
NQ = 4                    # SWDGE queues
GSPLIT = 2                # gathers per block (queue parallelism)

BF16 = mybir.dt.bfloat16
F32 = mybir.dt.float32
I16 = mybir.dt.int16


def _wrap_idx(flat):
    """Index stream -> [128, n/16] int16 tile (16-partition wrap, x8)."""
    return np.tile(flat.astype(np.int16).reshape(-1, 16).T, (8, 1)).copy()


def make_plan(input_labels, pos_labels, neg_labels, hierarchy_pairs,
              w_in, w_out):
    il = np.asarray(input_labels).astype(np.int64)
    pl = np.asarray(pos_labels).astype(np.int64)
    nl = np.asarray(neg_labels).astype(np.int64)
    hp = np.asarray(hierarchy_pairs).astype(np.int64)

    rlab = np.concatenate([pl, nl], axis=1)  # [B, 60]

    uniq = []  # [(core, sub)] -> unique label array
    for k in range(NCORES):
        for s in range(NSUB):
            r0 = k * BL + s * SUB
            uniq.append(np.unique(rlab[r0 : r0 + SUB]))
    capu = -(-max(len(u) for u in uniq) // 16) * 16

    per_core = []
    for k in range(NCORES):
        rows = slice(k * BL, (k + 1) * BL)
        iu, iinv = np.unique(il[rows], return_inverse=True)
        itab = np.zeros((BL, D), ml_dtypes.bfloat16)
        itab[: len(iu)] = w_in[iu]

        rtabs = []
        ridx = np.empty((BL, SLOTS), np.int16)
        for s in range(NSUB):
            u = uniq[k * NSUB + s]
            inv = np.searchsorted(u, rlab[k * BL + s * SUB : k * BL + (s + 1) * SUB])
            rt = np.zeros((capu, D), ml_dtypes.bfloat16)
            rt[: len(u)] = w_out[u]
            rtabs.append(rt)
            ridx[s * SUB : (s + 1) * SUB] = inv

        hl = hp[k * HLC : (k + 1) * HLC]  # [512, 2]
        hu, hinv = np.unique(hl, return_inverse=True)
        hinv = hinv.reshape(HLC, 2)
        htab = np.zeros((1024, D), ml_dtypes.bfloat16)
        htab[: len(hu)] = w_in[hu]

        # right stream: g = b*7680 + j*128 + p  ->  ridx[b*128+p, j]
        rstream = ridx.reshape(NBLK, P, SLOTS).transpose(0, 2, 1).reshape(-1)
        hstream = np.concatenate([hinv[:, 0], hinv[:, 1]])

        per_core.append({
            **{f"rt{s}": rtabs[s] for s in range(NSUB)},
            "itab": itab,
            "htab": htab,
            "ridx": _wrap_idx(rstream),
            "iidx": _wrap_idx(iinv),
            "hidx": _wrap_idx(hstream),
        })
    return capu, per_core


def build_program(capu, enable_asserts=False):
    nc = bacc.Bacc(
        "TRN2",
        target_bir_lowering=False,
        debug=False,
        enable_asserts=enable_asserts,
        num_devices=NCORES,
        num_swdge_queues=NQ,
    )

    rt = [
        nc.dram_tensor(f"rt{s}", [capu, D], BF16, kind="ExternalInput").ap()
        for s in range(NSUB)
    ]
    itab = nc.dram_tensor("itab", [BL, D], BF16, kind="ExternalInput").ap()
    htab = nc.dram_tensor("htab", [1024, D], BF16, kind="ExternalInput").ap()
    ridx_d = nc.dram_tensor("ridx", [P, NBLK * SLOTS * P // 16], I16,
                            kind="ExternalInput").ap()
    iidx_d = nc.dram_tensor("iidx", [P, BL // 16], I16,
                            kind="ExternalInput").ap()
    hidx_d = nc.dram_tensor("hidx", [P, 2 * HLC // 16], I16,
                            kind="ExternalInput").ap()
    out_d = nc.dram_tensor("out", [P, 2], F32, kind="ExternalOutput").ap()

    IDXB = SLOTS * P // 16  # idx cols per block (480)

    with tile.TileContext(nc) as tc:
        with (
            tc.tile_pool(name="idx", bufs=1) as idxp,
            tc.tile_pool(name="inp", bufs=1) as inpp,
            tc.tile_pool(name="gath", bufs=3) as gp,
            tc.tile_pool(name="prod", bufs=2) as prodp,
            tc.tile_pool(name="s1", bufs=2) as s1p,
            tc.tile_pool(name="dots", bufs=1) as dotsp,
            tc.tile_pool(name="end", bufs=1) as endp,
        ):
            ridx = idxp.tile([P, NBLK * IDXB], I16)
            nc.sync.dma_start(ridx[:], ridx_d)
            iidx = idxp.tile([P, BL // 16], I16)
            nc.sync.dma_start(iidx[:], iidx_d)
            hidx = idxp.tile([P, 2 * HLC // 16], I16)
            nc.sync.dma_start(hidx[:], hidx_d)

            itile = inpp.tile([P, NBLK, D], BF16)
            nc.gpsimd.dma_gather(
                itile[:], itab, iidx[:], BL, BL, D,
                single_packet=False, queue_num=0,
            )
            htile = inpp.tile([P, 2 * HLC // P, D], BF16)
            nc.gpsimd.dma_gather(
                htile[:], htab, hidx[:], 2 * HLC, 2 * HLC, D,
                single_packet=False, queue_num=1,
            )

            dots = dotsp.tile([P, NBLK, SLOTS], F32)
            qctr = [2]

            for b in range(NBLK):
                s = b // BPS
                g = gp.tile([P, SLOTS, D], BF16, tag="g")
                ns = SLOTS // GSPLIT
                for h in range(GSPLIT):
                    nc.gpsimd.dma_gather(
                        g[:, h * ns : (h + 1) * ns, :],
                        rt[s],
                        ridx[:, b * IDXB + h * ns * 8 : b * IDXB + (h + 1) * ns * 8],
                        ns * P, ns * P, D,
                        single_packet=False,
                        queue_num=qctr[0] % NQ,
                    )
                    qctr[0] += 1
                prod = prodp.tile([P, SLOTS, D], BF16, tag="prod")
                nc.vector.tensor_tensor(
                    out=prod[:],
                    in0=g[:],
                    in1=itile[:, b : b + 1, :].broadcast_to([P, SLOTS, D]),
                    op=mybir.AluOpType.mult,
                )
                s1 = s1p.tile([P, SLOTS, D // 2], BF16, tag="s1")
                nc.vector.tensor_tensor(
                    out=s1[:],
                    in0=prod[:, :, 0 : D // 2],
                    in1=prod[:, :, D // 2 : D],
                    op=mybir.AluOpType.add,
                )
                nc.vector.reduce_sum(
                    out=dots[:, b, :], in_=s1[:], axis=mybir.AxisListType.X
                )

            # hierarchy: htile slots 0:4 = left rows, 4:8 = right rows
            nh = HLC // P  # 4
            dif = endp.tile([P, nh, D], BF16)
            nc.vector.tensor_tensor(
                out=dif[:], in0=htile[:, 0:nh, :], in1=htile[:, nh : 2 * nh, :],
                op=mybir.AluOpType.subtract,
            )
            sq = endp.tile([P, nh, D], F32)
            nc.scalar.activation(
                out=sq[:], in_=dif[:],
                func=mybir.ActivationFunctionType.Square,
            )
            h_acc = endp.tile([P, 1], F32)
            nc.vector.reduce_sum(out=h_acc[:], in_=sq[:], axis=mybir.AxisListType.XY)

            # softplus(v) = max(v,0) + ln(1+exp(-|v|));
            # v = -dot for pos slots (j<10), +dot for neg slots.
            eb = endp.tile([P, 2, NBLK, SLOTS], F32)
            nc.vector.tensor_scalar(
                out=eb[:, 0, :, 0:C], in0=dots[:, :, 0:C],
                scalar1=0.0, scalar2=-1.0,
                op0=mybir.AluOpType.min, op1=mybir.AluOpType.mult,
            )
            nc.vector.tensor_scalar(
                out=eb[:, 0, :, C:SLOTS], in0=dots[:, :, C:SLOTS],
                scalar1=0.0, scalar2=None, op0=mybir.AluOpType.max,
            )
            absv = endp.tile([P, NBLK, SLOTS], F32)
            nc.scalar.activation(
                out=absv[:], in_=dots[:],
                func=mybir.ActivationFunctionType.Abs,
            )
            expv = endp.tile([P, NBLK, SLOTS], F32)
            nc.scalar.activation(
                out=expv[:], in_=absv[:],
                func=mybir.ActivationFunctionType.Exp, scale=-1.0,
            )
            nc.scalar.activation(
                out=eb[:, 1, :, :], in_=expv[:],
                func=mybir.ActivationFunctionType.Ln, bias=1.0,
            )
            r1 = endp.tile([P, 2, NBLK], F32)
            nc.vector.reduce_sum(out=r1[:], in_=eb[:], axis=mybir.AxisListType.X)
            s_acc = endp.tile([P, 1], F32)
            nc.vector.reduce_sum(out=s_acc[:], in_=r1[:], axis=mybir.AxisListType.XY)

            out_sb = endp.tile([P, 2], F32)
            nc.vector.tensor_copy(out_sb[:, 0:1], s_acc[:])
            nc.vector.tensor_copy(out_sb[:, 1:2], h_acc[:])
            nc.sync.dma_start(out_d, out_sb[:])

    nc.compile()
    return nc


def prepare(input_labels, pos_labels, neg_labels, hierarchy_pairs,
            in_embed_w, out_embed_w):
    w_in = np.asarray(in_embed_w, dtype=np.float32).astype(ml_dtypes.bfloat16)
    w_out = np.asarray(out_embed_w, dtype=np.float32).astype(ml_dtypes.bfloat16)

    capu, per_core = make_plan(input_labels, pos_labels, neg_labels,
                               hierarchy_pairs, w_in, w_out)
    nc = build_program(capu)
    return nc, per_core, None


def combine_results(per_core_outs, pads):
    s_total = 0.0
    h_total = 0.0
    for r in per_core_outs:
        o = r["out"].astype(np.float64)
        s_total += o[:, 0].sum()
        h_total += o[:, 1].sum()
    loss_graph = s_total / B
    loss_h = 0.5 * 1e-8 * h_total
    return (np.float32(loss_graph + loss_h), np.float32(loss_h))


def run_on_hw(nc, in_maps, **kwargs):
    from concourse.bass_utils import run_bass_kernel_spmd

    return run_bass_kernel_spmd(
        nc, in_maps, core_ids=list(range(NCORES)), **kwargs
    )


def kernel(input_labels, pos_labels, neg_labels, hierarchy_pairs,
           in_embed_w, out_embed_w):
    nc, in_maps, pads = prepare(
        input_labels, pos_labels, neg_labels, hierarchy_pairs,
        in_embed_w, out_embed_w,
    )
    res = run_on_hw(nc, in_maps)
    return combine_results(res.results, pads)
